# revision 28
# baseline (speedup 1.0000x reference)
"""Trainium2 Bass kernel for nn_CausalSelfAttention_22703197127379.

Reference computation (k/v are dead code — attention is stubbed to RoPE(q)):
    q    = hidden @ w_qkv[:, :4096]           # [8192, 4096]
    qr   = rope_neox(q, positions)            # per-head rotate-half RoPE
    out  = qr @ w_o                           # [8192, 4096]

Distribution: data-parallel over tokens — core c owns rows c*1024..(c+1)*1024.
No collectives; host concatenates the 8 shards.

Design (PE-bound at the bf16 roofline: 2.097M moving-rows/core @ 2.4 GHz =
874 us; everything else must hide under the matmul stream):
  * all matmul operands bf16 (PSUM accumulation stays f32): 3x less DMA +
    SBUF traffic than f32r; rel-err ~3e-3 vs the 2e-2 gate.
  * q stays resident in SBUF between the two matmuls — no qT DRAM bounce.
  * startup: the first ILV heads are interleaved over the contraction blocks
    so PE consumption (~1.3 us/eb for 3 heads) never outruns the xT DMA
    stream (~0.9 us/eb); DMA issue order is hand-interleaved (tiny first
    chunks -> first matmul starts after ~160KB, wq chunks woven into the
    xb stream, rope tables split and placed late).
  * phase-2 wo tiles for fq=0 prefetched near the end of phase 1 so the
    phase boundary has no DMA wait.
  * phase-2 PSUM drains alternate DVE/ACT so the final-8-bank drain tail
    halves.
  * RoPE reads each PSUM group once (copy to SBUF), math runs from SBUF;
    rotate-half via partition-swap DMA with signs folded into the host-built
    sin table.
  * post-schedule BIR pass drops LDWEIGHTS whose stationary matches the
    previous one (harmless; trims the instruction stream).

Per-core device kernel:
  phase 1: for each head h: Q^T[h] = sum_eb wq[eb,h].T @ x[eb] in PSUM;
           RoPE applied as qt = ps*C + swap_halves(ps*S), cast to bf16 into
           the persistent qt[h] SBUF tile.
  phase 2: outT[f,t] = sum_h wo[h,f].T @ qt[h], PSUM-accumulated over all
           32 head blocks, written transposed; host transposes back.
"""

import sys

if "/opt/trn_rl_repo" not in sys.path:
    sys.path.insert(0, "/opt/trn_rl_repo")

import numpy as np
import ml_dtypes

BF16_NP = ml_dtypes.bfloat16

NCORES = 8
T, E, QS = 8192, 4096, 4096
TL = T // NCORES          # 1024 tokens per core
NH = 32                   # q heads
HD = 128                  # head dim
HALF = HD // 2
EB = E // 128             # 32 contraction blocks
QB = QS // 128            # 32 head blocks
ROPE_THETA = 10000.0

_NC_CACHE = {}

TUNE = {
    "dedupe": True,
    "ilv": 3,             # heads interleaved during the xT stream
    "wq_ring": 4,         # wq tile ring size (>= ilv + 1)
    "wq_split": 2,        # DMAs per sequential wq head tile
    "ps1_bufs": 6,
    "rope_bufs": 2,
    "wop_bufs": 6,        # ring of 2-head [128,1024] wo tiles
    "ost_bufs": 4,
    "wo_prefetch": 5,     # fq0 wo pair-tiles issued near end of phase 1
    "wo_prefetch_at": 24, # ...right before this head's wq load
    "tab_at": (5, 6),     # xb-stream block after which each table half lands
    "drain_act": True,    # alternate PSUM drains between DVE and ACT
}


def _build_nc(loop_iters=None, timing=False):
    """Build the per-core NEFF. loop_iters wraps the compute body in a
    hardware For_i loop (timing-only builds; data goes stale after iter 0).
    timing=True swaps all I/O parameters for internal DRAM tensors (garbage
    contents, nothing shipped over the tunnel) plus a tiny sink output."""
    import contextlib

    import concourse.bacc as bacc
    import concourse.mybir as mybir
    from concourse.tile import TileContext

    F32 = mybir.dt.float32
    BF16 = mybir.dt.bfloat16

    nc = bacc.Bacc()
    # all inputs arrive pre-rearranged on host so every DMA is contiguous:
    # xT[p, eb*TL + t]            = bf16(hidden_shard.T)[eb*128 + p, t]
    # wq[h*128 + p, eb*HD + f]    = bf16(w_q)[eb*128 + p, h*HD + f]
    # wo[(fq*QB + h)*128 + p, f]  = bf16(w_o)[h*128 + p, fq*512 + f]
    if timing:
        def param(name, shape, dt, isOutput=False):
            return nc.dram_tensor(name, shape, dt)
    else:
        param = nc.declare_dram_parameter
    xT = param("xT", [128, EB * TL], BF16, isOutput=False)
    wq = param("wq", [NH * 128, EB * HD], BF16, isOutput=False)
    wo = param("wo", [(E // 512) * QB * 128, 512], BF16, isOutput=False)
    Ct = param("Ct", [HD, TL], F32, isOutput=False)
    St = param("St", [HD, TL], F32, isOutput=False)
    outT = param("outT", [E, TL], BF16, isOutput=True)
    sink = (nc.declare_dram_parameter("sink", [128, 16], BF16, isOutput=True)
            if timing else None)

    with TileContext(nc) as tc:
        loop_cm = (tc.For_i(0, loop_iters, 1) if loop_iters
                   else contextlib.nullcontext())
        with loop_cm:
            _emit_body(nc, tc, mybir, xT, wq, wo, Ct, St, outT)
        if timing:
            nc.sync.dma_start(out=sink[:], in_=outT[0:128, 0:16])

    nc.finalize()
    if TUNE["dedupe"]:
        _dedupe_ldweights(nc)
    return nc


def _dedupe_ldweights(nc):
    """Post-schedule BIR pass: drop an InstLdweights when its stationary
    access pattern is identical to the previous kept one, it carries no
    semaphores, and only wait-free InstMatmults sit between them (a waiting
    matmul could subsume a semaphore signalling a rewrite of the weights
    region).  Any other PE instruction resets the reference.  Safe because
    walrus MATMUL uses the persistently-loaded stationary operand."""
    def _sync_empty(inst):
        si = inst.sync_info
        return si is None or (not si.on_wait and not si.on_update)

    def _no_waits(inst):
        si = inst.sync_info
        return si is None or not si.on_wait

    for fn in nc.m.functions:
        for blk in fn.blocks:
            insts = blk.instructions
            ref_ap = None
            clean = True
            to_del = []
            for idx, inst in enumerate(insts):
                tname = type(inst).__name__
                if str(inst.engine) != "EngineType.PE":
                    continue
                if tname == "InstLdweights":
                    ap = str(inst.ins[0])
                    if ap == ref_ap and clean and _sync_empty(inst):
                        to_del.append(idx)
                    else:
                        ref_ap = ap
                        clean = True
                elif tname == "InstMatmult":
                    if not _no_waits(inst):
                        clean = False
                else:
                    ref_ap = None
                    clean = True
            for idx in reversed(to_del):
                del insts[idx]


def _emit_body(nc, tc, mybir, xT, wq, wo, Ct, St, outT):
    F32 = mybir.dt.float32
    BF16 = mybir.dt.bfloat16
    ILV = TUNE["ilv"]
    NW = max(TUNE["wq_ring"], ILV + 1)  # wq tile ring (tags cycled h % NW)

    with tc.tile_pool(name="xp", bufs=1) as xp, \
         tc.tile_pool(name="qtp", bufs=1) as qtp, \
         tc.tile_pool(name="wqp", bufs=1) as wqp, \
         tc.tile_pool(name="tab", bufs=1) as tab, \
         tc.tile_pool(name="rope", bufs=TUNE["rope_bufs"]) as rope, \
         tc.tile_pool(name="wop", bufs=TUNE["wop_bufs"]) as wop, \
         tc.tile_pool(name="ost", bufs=TUNE["ost_bufs"]) as ost:
        # persistent per-head RoPE'd q (bf16) — phase-2 moving operand
        qt = [qtp.tile([128, TL], BF16, tag=f"qt{h}", name=f"qt{h}")
              for h in range(NH)]
        # xb tiles (persistent, one per contraction block)
        xb = [xp.tile([128, TL], BF16, tag=f"xb{eb}", name=f"xb{eb}")
              for eb in range(EB)]
        # wq tile ring
        wqt = [wqp.tile([128, EB * HD], BF16, tag=f"wqh{i}", name=f"wqh{i}")
               for i in range(NW)]
        ct = tab.tile([HD, TL], F32, tag="ct")
        stt = tab.tile([HD, TL], F32, tag="st")

        # ---------------- phase 1 DMA prologue (issue order == priority) ----
        CW = 512              # wq chunk width (cols; covers 4 ebs)

        def wq_chunk(h, c0, c1):
            t = wqt[h % NW]
            nc.sync.dma_start(out=t[:, c0:c1],
                              in_=wq[h * 128:(h + 1) * 128, c0:c1])

        def xb_load(eb, c0=0, c1=TL):
            nc.sync.dma_start(out=xb[eb][:, c0:c1], in_=xT[:, eb * TL + c0:
                                                           eb * TL + c1])

        # first chunks unblock LDW(h0,eb0) + MM(h0,eb0,tch0) fast; weight
        # chunks go ahead of the same-rank xb so each head's first LDW is
        # ready when the interleaved stream reaches it
        wq_chunk(0, 0, CW)
        xb_load(0, 0, 512)
        xb_load(0, 512, TL)
        nxt = 1
        for h in range(1, ILV):
            wq_chunk(h, 0, CW)
            xb_load(nxt)
            nxt += 1
        while nxt < 4:
            xb_load(nxt)
            nxt += 1
        ta, tb = TUNE["tab_at"]
        for i in range(1, EB * HD // CW):        # chunk blocks 1..7
            for h in range(ILV):
                wq_chunk(h, i * CW, (i + 1) * CW)
            for eb in range(4 * i, min(4 * i + 4, EB)):
                xb_load(eb)
            if i == ta:
                nc.sync.dma_start(out=ct[:, 0:512], in_=Ct[:, 0:512])
                nc.sync.dma_start(out=stt[:, 0:512], in_=St[:, 0:512])
            if i == tb:
                nc.sync.dma_start(out=ct[:, 512:TL], in_=Ct[:, 512:TL])
                nc.sync.dma_start(out=stt[:, 512:TL], in_=St[:, 512:TL])
        # head ILV's weights follow the stream (fresh ring slot, needed
        # right after the interleaved group finishes)
        nsp = TUNE["wq_split"]
        cs = (EB * HD) // nsp
        for i in range(nsp):
            wq_chunk(ILV, i * cs, (i + 1) * cs)

        preloaded = {}

        def rope_drain(h, pss):
            for tch in range(2):
                ps = pss[tch]
                sl = slice(tch * 512, tch * 512 + 512)
                q0 = rope.tile([128, 512], F32, tag="q0")
                u = rope.tile([128, 512], F32, tag="u")
                qs = rope.tile([128, 512], F32, tag="qs")
                v = rope.tile([128, 512], F32, tag="v")
                # single PSUM read per group; math runs from SBUF. tch1's
                # PSUM read goes to ACT so both banks drain in parallel —
                # the ps1->ps2 pool-release barrier waits on these.
                if tch == 1:
                    nc.scalar.activation(q0[:], ps[:],
                                         mybir.ActivationFunctionType.Copy)
                else:
                    nc.vector.tensor_copy(q0[:], ps[:])
                nc.vector.tensor_mul(u[:], q0[:], stt[:, sl])
                nc.vector.tensor_mul(qs[:], q0[:], ct[:, sl])
                # rotate-half: v = swap_halves(u) via partition-offset DMA
                # (on the ACT HWDGE queue — keeps the SP queue's 625ns/desc
                # budget for the weight/activation streams)
                nc.sync.dma_start(out=v[0:HALF, :], in_=u[HALF:HD, :])
                nc.sync.dma_start(out=v[HALF:HD, :], in_=u[0:HALF, :])
                nc.vector.tensor_add(qt[h][:, sl], qs[:], v[:])

        # ---------------- phase 1: Q^T per head + RoPE ----------------------
        with tc.tile_pool(name="ps1", bufs=TUNE["ps1_bufs"],
                          space="PSUM") as ps1:
            # interleaved leading group: consumption of xb stays behind DMA
            grp = list(range(ILV))
            pss = {h: [ps1.tile([128, 512], F32, tag="ps1",
                                name=f"ps1_{h}_{i}") for i in range(2)]
                   for h in grp}
            for eb in range(EB):
                for h in grp:
                    for tch in range(2):
                        nc.tensor.matmul(
                            pss[h][tch][:],
                            wqt[h % NW][:, eb * HD:(eb + 1) * HD],
                            xb[eb][:, tch * 512:(tch + 1) * 512],
                            start=(eb == 0), stop=(eb == EB - 1),
                        )
            for h in grp:
                rope_drain(h, pss[h])

            # remaining heads sequential
            for h in range(ILV, NH):
                if h == TUNE["wo_prefetch_at"]:
                    for j in range(TUNE["wo_prefetch"]):
                        woh = wop.tile([128, 1024], BF16, tag="woh",
                                       name=f"woh_pre{j}")
                        nc.sync.dma_start(
                            out=woh[:].rearrange("p (two c) -> p two c",
                                                 two=2),
                            in_=wo[j * 256:(j + 1) * 256, :].rearrange(
                                "(two p) c -> p two c", two=2))
                        preloaded[j] = woh
                if h > ILV:  # head ILV's chunks were issued in the prologue
                    for i in range(nsp):
                        wq_chunk(h, i * cs, (i + 1) * cs)
                ph = [ps1.tile([128, 512], F32, tag="ps1",
                               name=f"ps1_{h}_{i}") for i in range(2)]
                for eb in range(EB):
                    for tch in range(2):
                        nc.tensor.matmul(
                            ph[tch][:],
                            wqt[h % NW][:, eb * HD:(eb + 1) * HD],
                            xb[eb][:, tch * 512:(tch + 1) * 512],
                            start=(eb == 0), stop=(eb == EB - 1),
                        )
                rope_drain(h, ph)

        # ---------------- phase 2: outT = sum_h wo[h].T @ qt[h] -------------
        # PSUM as four 2-bank tiles (one per fb); each drains with a single
        # wide f32->bf16 copy (alternating DVE/ACT) and a single wide DMA
        # (alternating SP/ACT HWDGE queues) — 4 descriptors per fq, not 8.
        with tc.tile_pool(name="ps2", bufs=4, space="PSUM") as ps2:
            for fq in range(E // 512):
                pss = [ps2.tile([128, 1024], F32, tag="ps2",
                                name=f"pss_{fq}_{fb}") for fb in range(4)]
                last = fq == E // 512 - 1
                for j in range(QB // 2):
                    # w_o loaded two heads per descriptor: partition p holds
                    # [head 2j row p (512) | head 2j+1 row p (512)]
                    r0 = (fq * QB + 2 * j) * 128
                    if fq == 0 and j in preloaded:
                        woh = preloaded[j]
                    else:
                        woh = wop.tile([128, 1024], BF16, tag="woh")
                        nc.sync.dma_start(
                            out=woh[:].rearrange("p (two c) -> p two c",
                                                 two=2),
                            in_=wo[r0:r0 + 256, :].rearrange(
                                "(two p) c -> p two c", two=2))
                    for hh in range(2):
                        h = 2 * j + hh
                        # in the very last head of the last fq, finish banks
                        # in reverse order so the tail drains pipeline
                        fbs = (range(3, -1, -1) if last and h == QB - 1
                               else range(4))
                        for fb in fbs:
                            for t2 in range(2):
                                nc.tensor.matmul(
                                    pss[fb][:, t2 * 512:(t2 + 1) * 512],
                                    woh[:, hh * 512 + fb * 128:
                                        hh * 512 + (fb + 1) * 128],
                                    qt[h][:, t2 * 512: t2 * 512 + 512],
                                    start=(h == 0), stop=(h == QB - 1),
                                )
                # wide 2-bank drains, split across DVE and ACT; one wide
                # outT DMA per fb keeps the shared-HWDGE descriptor count low
                dr = [3, 2, 1, 0] if last else [0, 1, 2, 3]
                for i, fb in enumerate(dr):
                    o = ost.tile([128, TL], BF16, tag="ost")
                    r = slice(fq * 512 + fb * 128, fq * 512 + (fb + 1) * 128)
                    if TUNE["drain_act"] and (i % 2 == 0 if last
                                              else fb % 2 == 1):
                        nc.scalar.activation(
                            o[:], pss[fb][:],
                            mybir.ActivationFunctionType.Copy)
                    else:
                        nc.vector.tensor_copy(o[:], pss[fb][:])
                    nc.sync.dma_start(out=outT[r, :], in_=o[:])


def _get_nc(loop_iters=None, timing=False):
    key = ("nc", loop_iters, timing)
    if key not in _NC_CACHE:
        _NC_CACHE[key] = _build_nc(loop_iters, timing)
    return _NC_CACHE[key]


def _rope_tables(positions):
    # mirrors reference fp32 math: inv_freq f32, freqs f32, cos/sin f32
    half = np.float32(HALF)
    inv_freq = (1.0 / (ROPE_THETA ** (np.arange(HALF, dtype=np.float32) / half))
                ).astype(np.float32)
    freqs = positions.astype(np.float32)[:, None] * inv_freq[None, :]  # [T, 64]
    cos = np.cos(freqs).astype(np.float32)
    sin = np.sin(freqs).astype(np.float32)
    # qT layout tables: Ct[d, t] = cos[t, d%64]
    # St[d, t] = +sin[t, d] for d<64, -sin[t, d-64] for d>=64, so that
    # qs + swap_halves(q * St) == neox rope of q.
    Ct = np.concatenate([cos.T, cos.T], axis=0)    # [128, T]
    St = np.concatenate([sin.T, -sin.T], axis=0)   # [128, T]
    return np.ascontiguousarray(Ct), np.ascontiguousarray(St)


def build_in_maps(hidden_states, positions, w_qkv, w_o):
    hidden = np.asarray(hidden_states, dtype=np.float32)
    pos = np.asarray(positions)
    wq_nat = np.asarray(w_qkv, dtype=np.float32)[:, :QS]
    wo_nat = np.asarray(w_o, dtype=np.float32)
    # pre-rearranged layouts (see _build_nc comments), cast to bf16 on host
    wq = np.ascontiguousarray(
        wq_nat.reshape(EB, 128, NH, HD).transpose(2, 1, 0, 3)
        .reshape(NH * 128, EB * HD).astype(BF16_NP))
    wo = np.ascontiguousarray(
        wo_nat.reshape(QB, 128, E // 512, 512).transpose(2, 0, 1, 3)
        .reshape((E // 512) * QB * 128, 512).astype(BF16_NP))
    Ct, St = _rope_tables(pos)
    in_maps = []
    for c in range(NCORES):
        sl = slice(c * TL, (c + 1) * TL)
        xTc = np.ascontiguousarray(
            hidden[sl].T.reshape(EB, 128, TL).transpose(1, 0, 2)
            .reshape(128, EB * TL).astype(BF16_NP))
        in_maps.append({
            "xT": xTc,
            "wq": wq,
            "wo": wo,
            "Ct": np.ascontiguousarray(Ct[:, sl]),
            "St": np.ascontiguousarray(St[:, sl]),
        })
    return in_maps


def kernel(hidden_states, positions, w_qkv, w_o):
    from concourse.bass_utils import run_bass_kernel_spmd

    nc = _get_nc()
    in_maps = build_in_maps(hidden_states, positions, w_qkv, w_o)
    res = run_bass_kernel_spmd(nc, in_maps, core_ids=list(range(NCORES)))
    out = np.concatenate(
        [np.asarray(res.results[c]["outT"]).astype(np.float32).T
         for c in range(NCORES)], axis=0)
    return np.ascontiguousarray(out)
